# revision 50
# baseline (speedup 1.0000x reference)
"""Trainium2 Bass kernel for nn_AlignModel.

Computes out[b, j, i] = sigmoid(simp[b,j]·w_s + orig[b,i]·w_o + bias) where
orig/simp are the two halves of prop_state[b] ([B, 2S, D] -> [B,S,D] each),
w_o = W[0,:D], w_s = W[0,D:].

Sharding: data-parallel over batch B=8 across the 8 NeuronCores.  Host-side
staging per core (layout only -- all compute is on device):
  xot  [512, 2048] f16  = orig(b).T          (d-major, so PE can contract d)
  xs   [2048, 512] f16  = simp(b)
  wrep [128, 4, 128] f16: wrep[k,e,m] = w_o[e*128+k]  (stationary replicated
        along the PE output dim -> matmul broadcasts s_o to all partitions)
  wsbc [128, 512]  f16  = w_s replicated across partitions
  out  [2048, 2048] f16, host-upcast to f32.

The 2e-2 rel-err gate admits fp16 end to end (sigmoid outputs in (0,1):
~5e-4 rel err; fp16-input dots with f32 accumulation: ~1e-4 score error).
Per-core HBM traffic: 4.45 MiB in + 8.39 MiB out.

Engine schedule (from trace iteration; engines run disjoint jobs):
  - PE: psum_so[p,i] = b + sum_d w_o[d]*orig[i,d] via 4 bias seeds + 16
    K=128/N=512 fp16 matmuls (w_rep stationary).  s_o never materializes;
    the matmul does the reduction AND the 128-row broadcast.
  - DVE: simp dots only (fp16 mul at 2x + batched 4-tile reduce at 1x)
    into s_sb_mat columns -> always ahead of ScalarE's 2us/tile cadence.
  - ScalarE: ONLY the 16 sigmoid ACTIVATEs, [128,2048] PSUM->SBUF f16,
    bias port adds s_s[t*128+p].  ACT table preloaded by a dep-free dummy.
  - Load order on the sync queue (FIFO): xs group 0 -> xot e=0..3 -> xs
    groups 1-3; stores follow.  All per-partition descriptor lines are
    >=4KB except xs (1KB, layout-forced); small chunks measured
    ~100-150 GB/s vs ~400 GB/s at 4KB.
"""

import numpy as np

import concourse.mybir as mybir
from concourse import bacc, bass_utils
from concourse.tile import TileContext

P = 128          # partitions
D = 512          # feature dim
S = 2048         # sents
NT = S // P      # 16 row-tiles
NE = D // P      # 4 contraction chunks
SCH = 4          # simp tiles per load group
NSC = NT // SCH
OGROUPS = [1, 1, 2, 4, 4, 2, 1, 1]   # output row-tiles per store
NCORES = 8
F32 = mybir.dt.float32
F16 = mybir.dt.float16


def _kernel_body(tc, out, xot, xs, wrep, wsbc, bvec):
    nc = tc.nc
    # simp half: logical layout is partition-inner (j = n*P + p, so bias
    # columns drop out of the reduce), but the HOST pre-permutes rows so
    # HBM row p*NT+n holds simp row n*P+p -> per-partition descriptor
    # lines are 16KB contiguous (1KB lines measured ~250 GB/s vs ~400)
    xs_re = xs.rearrange("(p n) d -> p n d", n=NT)

    with (
        tc.tile_pool(name="consts", bufs=1) as cpool,
        tc.tile_pool(name="xin", bufs=1) as xpool,
        tc.tile_pool(name="scratch", bufs=3) as spool,
        tc.tile_pool(name="outbuf", bufs=1) as opool,
        tc.tile_pool(name="psum", bufs=1, space="PSUM") as ppool,
    ):
        # preload the sigmoid ACT table while DMAs run: dummy activation
        # whose only deps are two DVE memsets, so it issues almost at t=0.
        dummy = cpool.tile([1, 1], F32, tag="dummy")
        dummy_b = cpool.tile([1, 1], F32, tag="dummyb")
        nc.vector.memset(dummy, 0.0)
        nc.vector.memset(dummy_b, 0.0)
        nc.scalar.activation(dummy, dummy,
                             mybir.ActivationFunctionType.Sigmoid,
                             bias=dummy_b[:, 0:1])

        # tiny loads on the scalar HWDGE queue (empty early, so these land
        # well before the big sync-queue stream needs them); wrep and wsbc
        # ride in one contiguous [128, 1024] tensor (separate loads had
        # 256B descriptors and landed late enough to stall the first PE
        # matmul batch)
        # weights ride at the head of the sync queue (tiny); the scalar
        # HWDGE queue measurably delivered them ~4us late, stalling the
        # first PE batch behind an already-landed xot_0
        b_sb = cpool.tile([P, 1], F32, tag="bsb")
        wcat_sb = cpool.tile([P, NE * P + D], F16, tag="wcat")
        nc.sync.dma_start(out=wcat_sb, in_=wrep)
        nc.scalar.dma_start(out=b_sb, in_=bvec.broadcast_to([P, 1]))
        wrep_sb = wcat_sb[:, 0:NE * P].rearrange("p (e m) -> p e m", e=NE)
        wsbc_sb = wcat_sb[:, NE * P:NE * P + D]

        # --- input stream (sync queue, FIFO): 2 simp tiles (just enough to
        # un-gate sigmoid 0), xot e0..3 with e3 column-split (sigmoid 0's
        # first half starts when PSUM banks 0-1 finish), then simp rest ---
        xs_all = xpool.tile([P, NT, D], F16, tag="xs")
        nc.sync.dma_start(out=xs_all[:, 0:2, :], in_=xs_re[:, 0:2, :])
        xot_t = []
        for e in range(NE):
            xt = xpool.tile([P, S], F16, tag=f"xot{e}", name=f"xot{e}")
            nc.sync.dma_start(out=xt, in_=xot[e * P:(e + 1) * P, :])
            xot_t.append(xt)
        nc.sync.dma_start(out=xs_all[:, 2:SCH, :], in_=xs_re[:, 2:SCH, :])
        nc.sync.dma_start(out=xs_all[:, SCH:10, :], in_=xs_re[:, SCH:10, :])
        nc.sync.dma_start(out=xs_all[:, 10:NT, :], in_=xs_re[:, 10:NT, :])

        s_sb_mat = cpool.tile([P, NT], F32, tag="ssmat")  # s_s, col t
        sob_psum = ppool.tile([P, S], F32, tag="sob")     # b + s_o, all rows

        # --- PE: accumulate w_o-weighted transposed orig (the bias b is
        # folded into the s_s bias columns instead of a PSUM seed, so the
        # first batch fires the moment xot_0 lands) ---
        for e in range(NE):
            for j in range(S // 512):
                nc.tensor.matmul(sob_psum[:, j * 512:(j + 1) * 512],
                                 wrep_sb[:, e, :],
                                 xot_t[e][:, j * 512:(j + 1) * 512],
                                 start=(e == 0), stop=(e == NE - 1))

        # --- simp dots (DVE) + sigmoid row-blocks (ScalarE) + stores ---
        group_of_tile = []
        for gi, gsz in enumerate(OGROUPS):
            group_of_tile += [gi] * gsz
        group_start = np.cumsum([0] + OGROUPS).tolist()

        out_all = opool.tile([P, NT, S], F16, tag="oall")
        DGROUPS = [(0, 2), (2, 4), (4, 8), (8, 12), (12, 16)]
        for g, (lo, hi) in enumerate(DGROUPS):
            prod = spool.tile([P, SCH, D], F16, tag="prod", name=f"ps{g}")
            for blk in range(hi - lo):
                nc.vector.tensor_mul(out=prod[:, blk, :],
                                     in0=xs_all[:, lo + blk, :],
                                     in1=wsbc_sb)
            nc.vector.tensor_reduce(
                s_sb_mat[:, lo:hi], prod[:, 0:hi - lo, :],
                axis=mybir.AxisListType.X, op=mybir.AluOpType.add)
            nc.vector.tensor_scalar_add(s_sb_mat[:, lo:hi],
                                        s_sb_mat[:, lo:hi], b_sb)
            for blk in range(hi - lo):
                t = lo + blk
                nc.scalar.activation(
                    out_all[:, t, :], sob_psum,
                    mybir.ActivationFunctionType.Sigmoid,
                    bias=s_sb_mat[:, t:t + 1],
                    scale=1.0,
                )
                gi = group_of_tile[t]
                if t == group_start[gi] + OGROUPS[gi] - 1:
                    t0_g = group_start[gi]
                    gsz = OGROUPS[gi]
                    r0 = t0_g * P
                    if gsz == 1:
                        nc.sync.dma_start(out=out[r0:r0 + P, :],
                                          in_=out_all[:, t0_g, :])
                    else:
                        dst = out[r0:r0 + gsz * P, :].rearrange(
                            "(q p) i -> p q i", p=P)
                        nc.sync.dma_start(out=dst,
                                          in_=out_all[:, t0_g:t0_g + gsz, :])


def build_program():
    nc = bacc.Bacc(
        "TRN2",
        debug=False,
        target_bir_lowering=False,
        num_devices=NCORES,
    )
    xot = nc.dram_tensor("xot", [D, S], F16, kind="ExternalInput").ap()
    xs = nc.dram_tensor("xs", [S, D], F16, kind="ExternalInput").ap()
    wrep = nc.dram_tensor("wcat", [P, NE * P + D], F16,
                          kind="ExternalInput").ap()
    bvec = nc.dram_tensor("bvec", [1, 1], F32, kind="ExternalInput").ap()
    out = nc.dram_tensor("out", [S, S], F16, kind="ExternalOutput").ap()
    with TileContext(nc) as tc:
        _kernel_body(tc, out, xot, xs, wrep, None, bvec)
    nc.compile()
    return nc


_PROGRAM = None


def _get_program():
    global _PROGRAM
    if _PROGRAM is None:
        _PROGRAM = build_program()
    return _PROGRAM


def make_in_maps(prop_state, W, b):
    prop = np.asarray(prop_state, dtype=np.float32).astype(np.float16)
    w = np.asarray(W, dtype=np.float32).reshape(2 * D).astype(np.float16)
    w_o, w_s = w[:D], w[D:]
    # wcat = [wrep | wsbc]: wrep[k, e*128+m] = w_o[e*128+k] (stationary
    # replicated along the PE output dim m), wsbc = w_s on every partition
    wcat = np.empty((P, NE * P + D), dtype=np.float16)
    wcat[:, :NE * P] = np.repeat(w_o.reshape(NE, P).T, P, axis=1).reshape(
        P, NE * P)
    wcat[:, NE * P:] = w_s[None, :]
    wcat = np.ascontiguousarray(wcat)
    bv = np.ascontiguousarray(np.asarray(b, dtype=np.float32).reshape(1, 1))
    maps = []
    for i in range(NCORES):
        xot = np.ascontiguousarray(prop[i, :S].T)         # [512, 2048]
        # permute simp rows so HBM row p*NT+n = simp row n*P+p (contiguous
        # per-partition descriptor lines for the partition-inner layout)
        xs = np.ascontiguousarray(
            prop[i, S:].reshape(NT, P, D).transpose(1, 0, 2).reshape(S, D))
        maps.append({"xot": xot, "xs": xs, "wcat": wcat, "bvec": bv})
    return maps


def kernel(A, prop_state, W, b, _trace=False):
    nc = _get_program()
    in_maps = make_in_maps(prop_state, W, b)
    res = bass_utils.run_bass_kernel_spmd(
        nc, in_maps, core_ids=list(range(NCORES)), trace=_trace)
    out = np.stack([res.results[i]["out"] for i in range(NCORES)], axis=0)
    if _trace:
        kernel.last_results = res
    return out.astype(np.float32)


# revision 52
# speedup vs baseline: 1.2142x; 1.2142x over previous
"""Trainium2 Bass kernel for nn_AlignModel.

Computes out[b, j, i] = sigmoid(simp[b,j]·w_s + orig[b,i]·w_o + bias) where
orig/simp are the two halves of prop_state[b] ([B, 2S, D] -> [B,S,D] each),
w_o = W[0,:D], w_s = W[0,D:].

Sharding: data-parallel over batch B=8 across the 8 NeuronCores.  Host-side
staging per core (layout only -- all compute is on device):
  xot  [512, 2048] f16 = orig(b).T           (d-major, so PE can contract d)
  xs   [2048, 512] f16 = simp(b), rows permuted so HBM row p*16+n holds
        simp row n*128+p (partition-inner layout with contiguous >=4KB
        per-partition descriptor lines; 1KB lines measured ~2x slower)
  wcat [128, 1024] f16 = [wrep | wsbc]: wrep[k, e*128+m] = w_o[e*128+k]
        (stationary replicated along the PE output dim m, so the matmul
        broadcasts s_o to all partitions), wsbc = w_s on every partition
  out  [2048, 2048] f16, host-upcast to f32.

The 2e-2 rel-err gate admits fp16 end to end (sigmoid outputs in (0,1):
~5e-4 rel err; fp16-input dots with f32 accumulation: ~1e-4 score error;
measured 1.1e-3).  Per-core HBM traffic: 4.5 MiB in + 8.39 MiB out
(vs 24 MiB for the f32 baseline).

Engine schedule (from ~15 profiled iterations; engines run disjoint jobs):
  - PE: psum[p,i] = sum_d w_o[d]*orig[i,d] via 16 K=128/N=512 fp16
    matmuls (w_rep stationary).  s_o never materializes; the matmul does
    the reduction AND the 128-row broadcast.  The bias b is folded into
    the s_s columns (a PSUM b-seed would gate the first batch on a late
    b_row; with start=True on e==0 the PE fires as soon as xot_0 lands).
  - DVE: simp dots only (fp16 mul at 2x + batched reduce at 1x) into
    s_sb_mat columns + tiny +b adds -> ahead of ScalarE's 2us cadence.
  - ScalarE: ONLY the 16 sigmoid ACTIVATEs ((N+352)/1.2GHz each,
    [128,2048] PSUM->SBUF f16, bias port adds s_s[t*128+p] + b).  The
    ACT table is preloaded by a dep-free dummy at t~0 (lazy load cost
    ~1.3us; any Exp/Sigmoid interleaving would reload it per switch).
  - Load order on the sync queue (FIFO): wcat first (on the scalar HWDGE
    queue it landed ~4us late and stalled the first PE batch), 2 simp
    tiles (un-gates sigmoid 0), xot e0..3, simp rest; stores follow,
    geometric group sizes [1,1,2,4,4,2,1,1] so the first store ships
    after one sigmoid and the last store is small.
  - Rejected after measurement: offloading 4-6 row-blocks to a
    PE+DVE path via sigmoid(s) = 1/(1+e^-(s_o+b) e^-s_s) with
    reciprocal_approx_fast (v5-v9): the e^-(s_o+b) exp is a 1-lane 2.3us
    ScalarE op, the ACT table switch costs ~1.4us, SWDGE cast-stores ran
    ~5us each, and teardown grew -- measured consistently 3-10us WORSE
    than this simpler schedule.
"""

import numpy as np

import concourse.mybir as mybir
from concourse import bacc, bass_utils
from concourse.tile import TileContext

P = 128          # partitions
D = 512          # feature dim
S = 2048         # sents
NT = S // P      # 16 row-tiles
NE = D // P      # 4 contraction chunks
SCH = 4          # simp tiles per load group
NSC = NT // SCH
OGROUPS = [1, 1, 2, 4, 4, 2, 1, 1]   # output row-tiles per store
NCORES = 8
F32 = mybir.dt.float32
F16 = mybir.dt.float16


def _kernel_body(tc, out, xot, xs, wcat, bvec):
    nc = tc.nc
    # simp half: logical layout is partition-inner (j = n*P + p, so bias
    # columns drop out of the reduce), but the HOST pre-permutes rows so
    # HBM row p*NT+n holds simp row n*P+p -> per-partition descriptor
    # lines are 16KB contiguous (1KB lines measured ~250 GB/s vs ~400)
    xs_re = xs.rearrange("(p n) d -> p n d", n=NT)

    with (
        tc.tile_pool(name="consts", bufs=1) as cpool,
        tc.tile_pool(name="xin", bufs=1) as xpool,
        tc.tile_pool(name="scratch", bufs=3) as spool,
        tc.tile_pool(name="outbuf", bufs=1) as opool,
        tc.tile_pool(name="psum", bufs=1, space="PSUM") as ppool,
    ):
        # preload the sigmoid ACT table while DMAs run: dummy activation
        # whose only deps are two DVE memsets, so it issues almost at t=0.
        dummy = cpool.tile([1, 1], F32, tag="dummy")
        dummy_b = cpool.tile([1, 1], F32, tag="dummyb")
        nc.vector.memset(dummy, 0.0)
        nc.vector.memset(dummy_b, 0.0)
        nc.scalar.activation(dummy, dummy,
                             mybir.ActivationFunctionType.Sigmoid,
                             bias=dummy_b[:, 0:1])

        # tiny loads on the scalar HWDGE queue (empty early, so these land
        # well before the big sync-queue stream needs them); wrep and wsbc
        # ride in one contiguous [128, 1024] tensor (separate loads had
        # 256B descriptors and landed late enough to stall the first PE
        # matmul batch)
        # weights ride at the head of the sync queue (tiny); the scalar
        # HWDGE queue measurably delivered them ~4us late, stalling the
        # first PE batch behind an already-landed xot_0
        b_sb = cpool.tile([P, 1], F32, tag="bsb")
        wcat_sb = cpool.tile([P, NE * P + D], F16, tag="wcat")
        nc.sync.dma_start(out=wcat_sb, in_=wcat)
        nc.scalar.dma_start(out=b_sb, in_=bvec.broadcast_to([P, 1]))
        wrep_sb = wcat_sb[:, 0:NE * P].rearrange("p (e m) -> p e m", e=NE)
        wsbc_sb = wcat_sb[:, NE * P:NE * P + D]

        # --- input stream (sync queue, FIFO): 2 simp tiles (just enough to
        # un-gate sigmoid 0), xot e0..3 with e3 column-split (sigmoid 0's
        # first half starts when PSUM banks 0-1 finish), then simp rest ---
        xs_all = xpool.tile([P, NT, D], F16, tag="xs")
        nc.sync.dma_start(out=xs_all[:, 0:2, :], in_=xs_re[:, 0:2, :])
        xot_t = []
        for e in range(NE):
            xt = xpool.tile([P, S], F16, tag=f"xot{e}", name=f"xot{e}")
            nc.sync.dma_start(out=xt, in_=xot[e * P:(e + 1) * P, :])
            xot_t.append(xt)
        nc.sync.dma_start(out=xs_all[:, 2:SCH, :], in_=xs_re[:, 2:SCH, :])
        nc.sync.dma_start(out=xs_all[:, SCH:10, :], in_=xs_re[:, SCH:10, :])
        nc.sync.dma_start(out=xs_all[:, 10:NT, :], in_=xs_re[:, 10:NT, :])

        s_sb_mat = cpool.tile([P, NT], F32, tag="ssmat")  # s_s, col t
        sob_psum = ppool.tile([P, S], F32, tag="sob")     # b + s_o, all rows

        # --- PE: accumulate w_o-weighted transposed orig (the bias b is
        # folded into the s_s bias columns instead of a PSUM seed, so the
        # first batch fires the moment xot_0 lands) ---
        for e in range(NE):
            for j in range(S // 512):
                nc.tensor.matmul(sob_psum[:, j * 512:(j + 1) * 512],
                                 wrep_sb[:, e, :],
                                 xot_t[e][:, j * 512:(j + 1) * 512],
                                 start=(e == 0), stop=(e == NE - 1))

        # --- simp dots (DVE) + sigmoid row-blocks (ScalarE) + stores ---
        group_of_tile = []
        for gi, gsz in enumerate(OGROUPS):
            group_of_tile += [gi] * gsz
        group_start = np.cumsum([0] + OGROUPS).tolist()

        out_all = opool.tile([P, NT, S], F16, tag="oall")
        DGROUPS = [(0, 2), (2, 4), (4, 8), (8, 12), (12, 16)]
        for g, (lo, hi) in enumerate(DGROUPS):
            prod = spool.tile([P, SCH, D], F16, tag="prod", name=f"ps{g}")
            for blk in range(hi - lo):
                nc.vector.tensor_mul(out=prod[:, blk, :],
                                     in0=xs_all[:, lo + blk, :],
                                     in1=wsbc_sb)
            nc.vector.tensor_reduce(
                s_sb_mat[:, lo:hi], prod[:, 0:hi - lo, :],
                axis=mybir.AxisListType.X, op=mybir.AluOpType.add)
            nc.vector.tensor_scalar_add(s_sb_mat[:, lo:hi],
                                        s_sb_mat[:, lo:hi], b_sb)
            for blk in range(hi - lo):
                t = lo + blk
                nc.scalar.activation(
                    out_all[:, t, :], sob_psum,
                    mybir.ActivationFunctionType.Sigmoid,
                    bias=s_sb_mat[:, t:t + 1],
                    scale=1.0,
                )
                gi = group_of_tile[t]
                if t == group_start[gi] + OGROUPS[gi] - 1:
                    t0_g = group_start[gi]
                    gsz = OGROUPS[gi]
                    r0 = t0_g * P
                    if gsz == 1:
                        nc.sync.dma_start(out=out[r0:r0 + P, :],
                                          in_=out_all[:, t0_g, :])
                    else:
                        dst = out[r0:r0 + gsz * P, :].rearrange(
                            "(q p) i -> p q i", p=P)
                        nc.sync.dma_start(out=dst,
                                          in_=out_all[:, t0_g:t0_g + gsz, :])


def build_program():
    nc = bacc.Bacc(
        "TRN2",
        debug=False,
        target_bir_lowering=False,
        num_devices=NCORES,
    )
    xot = nc.dram_tensor("xot", [D, S], F16, kind="ExternalInput").ap()
    xs = nc.dram_tensor("xs", [S, D], F16, kind="ExternalInput").ap()
    wcat = nc.dram_tensor("wcat", [P, NE * P + D], F16,
                          kind="ExternalInput").ap()
    bvec = nc.dram_tensor("bvec", [1, 1], F32, kind="ExternalInput").ap()
    out = nc.dram_tensor("out", [S, S], F16, kind="ExternalOutput").ap()
    with TileContext(nc) as tc:
        _kernel_body(tc, out, xot, xs, wcat, bvec)
    nc.compile()
    return nc


_PROGRAM = None


def _get_program():
    global _PROGRAM
    if _PROGRAM is None:
        _PROGRAM = build_program()
    return _PROGRAM


def make_in_maps(prop_state, W, b):
    prop = np.asarray(prop_state, dtype=np.float32).astype(np.float16)
    w = np.asarray(W, dtype=np.float32).reshape(2 * D).astype(np.float16)
    w_o, w_s = w[:D], w[D:]
    # wcat = [wrep | wsbc]: wrep[k, e*128+m] = w_o[e*128+k] (stationary
    # replicated along the PE output dim m), wsbc = w_s on every partition
    wcat = np.empty((P, NE * P + D), dtype=np.float16)
    wcat[:, :NE * P] = np.repeat(w_o.reshape(NE, P).T, P, axis=1).reshape(
        P, NE * P)
    wcat[:, NE * P:] = w_s[None, :]
    wcat = np.ascontiguousarray(wcat)
    bv = np.ascontiguousarray(np.asarray(b, dtype=np.float32).reshape(1, 1))
    maps = []
    for i in range(NCORES):
        xot = np.ascontiguousarray(prop[i, :S].T)         # [512, 2048]
        # permute simp rows so HBM row p*NT+n = simp row n*P+p (contiguous
        # per-partition descriptor lines for the partition-inner layout)
        xs = np.ascontiguousarray(
            prop[i, S:].reshape(NT, P, D).transpose(1, 0, 2).reshape(S, D))
        maps.append({"xot": xot, "xs": xs, "wcat": wcat, "bvec": bv})
    return maps


def kernel(A, prop_state, W, b, _trace=False):
    nc = _get_program()
    in_maps = make_in_maps(prop_state, W, b)
    res = bass_utils.run_bass_kernel_spmd(
        nc, in_maps, core_ids=list(range(NCORES)), trace=_trace)
    out = np.stack([res.results[i]["out"] for i in range(NCORES)], axis=0)
    if _trace:
        kernel.last_results = res
    return out.astype(np.float32)
